# revision 16
# baseline (speedup 1.0000x reference)
"""MoE-routed DIAYN discriminator kernel for 8 Trainium2 NeuronCores.

Reference semantics: x = concat([graph, state, next_state], -1); for each
row, run the 3-layer MLP of the LAST factor i<NF with graph[:, i]==1
(rows with no active factor output 0). The dense reference computes all
NF expert MLPs for every row; we instead route each row to exactly one
expert on the host, pack rows into 8 SPMD shards, and run one dense
per-expert MLP stream per core.

Sharding: rows are grouped by expert into BLK-row blocks. Every core
executes the same static "profile" of G runs (run g = prof[g] blocks);
each run uses one weight set, supplied per-core as data. A small host-side
search picks (G, prof) and an assignment of runs -> experts that covers
the actual per-expert block counts with minimal padding + weight traffic.

Device kernel (per run, per block, activations kept transposed [feat, row]):
  h1 = relu(W1^T x + b1); h2 = relu(W2^T h1 + b2); out = W3^T h2 + b3
matmuls run in bf16 (same 1 row/cycle PE rate as fp32r but ~10ns less
per-MM overhead, LDWEIGHTS fully hidden, and half the DMA bytes); PSUM
accumulation, biases and the output stay fp32 (rel err ~4.5e-3 vs the
fp32 reference, well inside the 2e-2 gate).
Weights stream m-major ([128, KO, 128] chunk per output m-tile) so each
arriving chunk enables complete k-accumulation chains; the first three
blocks' chains are emitted m-interleaved so the in-order PE always has
runnable work while W1 streams in (x rides the scalar ring in parallel).
"""

import numpy as np

import concourse.bass as bass
import concourse.mybir as mybir
from concourse import bacc
from concourse.tile import TileContext
from concourse.bass_utils import run_bass_kernel_spmd

NCORES = 8
BLK = 272  # rows per matmul block; >=256 (f32r full rate), <=512 (PSUM bank)

F32 = mybir.dt.float32
BF16 = mybir.dt.bfloat16
NPBF16 = mybir.dt.np(BF16)

# Rough per-core cost weights for the plan search (ns).
_COST_BLOCK = int(152 * (BLK / 2.4 + 3))  # PE ns per block (152 matmuls)
_COST_RUN = 12_000  # partially-exposed weight-set DMA per extra run

_program_cache = {}


# ---------------------------------------------------------------- planning
def _compositions(total, parts):
    """Non-increasing positive integer compositions of `total` into `parts`."""
    if parts == 1:
        yield (total,)
        return
    for first in range((total + parts - 1) // parts, total - parts + 2):
        for rest in _compositions(total - first, parts - 1):
            if rest[0] <= first:
                yield (first,) + rest


def _try_assign(demands, prof):
    """Greedy cover of per-expert block demands by the 8x-replicated profile.

    demands: list of (n_blocks, expert) sorted desc. Returns dict
    run_size -> list of experts (8 entries per profile slot of that size,
    padding slots filled with the largest expert) or None if infeasible.
    """
    runs = sorted([t for t in prof for _ in range(NCORES)], reverse=True)
    used = []  # (size, expert)
    for n, e in demands:
        rem = n
        while rem > 0:
            if not runs:
                return None
            # largest run <= rem, else smallest run (minimal overshoot)
            pick = None
            for i, s in enumerate(runs):
                if s <= rem:
                    pick = i
                    break
            if pick is None:
                pick = len(runs) - 1
            s = runs.pop(pick)
            used.append((s, e))
            rem -= s
    pad_expert = demands[0][1]
    for s in runs:
        used.append((s, pad_expert))
    by_size = {}
    for s, e in used:
        by_size.setdefault(s, []).append(e)
    return by_size


def _make_plan(nblk):
    """nblk: per-expert block counts. Returns (prof, expert_of[core][g])."""
    demands = sorted(
        [(n, e) for e, n in enumerate(nblk) if n > 0], reverse=True
    )
    total = sum(n for n, _ in demands)
    mincap = (total + NCORES - 1) // NCORES
    best = None
    for G in range(1, 9):
        for cap in range(mincap, mincap + 6):
            for prof in _compositions(cap, G):
                a = _try_assign(demands, prof)
                if a is None:
                    continue
                cost = cap * _COST_BLOCK + G * _COST_RUN
                if best is None or cost < best[0]:
                    best = (cost, prof, a)
    assert best is not None, "no feasible run plan found"
    _, prof, by_size = best
    queues = {s: list(es) for s, es in by_size.items()}
    expert_of = [[None] * len(prof) for _ in range(NCORES)]
    for g, s in enumerate(prof):
        for core in range(NCORES):
            expert_of[core][g] = queues[s].pop(0)
    return list(prof), expert_of


# ---------------------------------------------------------------- device
def _build_program(prof, KO1, KO2, H, C, blk):
    """Build + compile the SPMD Bass program for a run profile."""
    key = (tuple(prof), KO1, KO2, H, C, blk)
    if key in _program_cache:
        return _program_cache[key]

    G = len(prof)
    NB = sum(prof)
    M1 = H // 128
    relu = mybir.ActivationFunctionType.Relu
    ident = mybir.ActivationFunctionType.Identity

    nc = bacc.Bacc("TRN2", target_bir_lowering=False, debug=False,
                   num_devices=NCORES)
    x_d = nc.dram_tensor("xb", [NB, 128, KO1, blk], BF16,
                         kind="ExternalInput").ap()
    # m-major weight layout: [G, 128, M1, KO, 128]
    w1_d = nc.dram_tensor("w1", [G, 128, M1, KO1, 128], BF16,
                          kind="ExternalInput").ap()
    w2_d = nc.dram_tensor("w2", [G, 128, M1, KO2, 128], BF16,
                          kind="ExternalInput").ap()
    w3_d = nc.dram_tensor("w3", [G, 128, KO2, C], BF16,
                          kind="ExternalInput").ap()
    b1_d = nc.dram_tensor("b1", [G, H], F32, kind="ExternalInput").ap()
    b2_d = nc.dram_tensor("b2", [G, H], F32, kind="ExternalInput").ap()
    b3_d = nc.dram_tensor("b3", [G, C], F32, kind="ExternalInput").ap()
    out_d = nc.dram_tensor("outb", [NB, C, blk], F32, kind="ExternalOutput").ap()

    runs = []
    for g, T in enumerate(prof):
        runs += [g] * T

    with TileContext(nc) as tc:
        with (
            tc.tile_pool(name="w", bufs=2) as wpool,
            tc.tile_pool(name="x", bufs=3) as xpool,
            tc.tile_pool(name="h1", bufs=4) as h1pool,
            tc.tile_pool(name="h2", bufs=1) as h2pool,
            tc.tile_pool(name="o", bufs=2) as opool,
            tc.tile_pool(name="ps", bufs=8, space="PSUM") as pspool,
        ):
            def emit_weights(g):
                # m-major stream on the sync ring: W1 m-chunks (each one
                # enables a full k-chain), biases early, then W2 m-chunks,
                # then W3.
                w1m, w2m = [], []
                for m in range(M1):
                    wt = wpool.tile([128, KO1, 128], BF16, tag=f"w1m{m}")
                    nc.sync.dma_start(wt[:], w1_d[g, :, m])
                    w1m.append(wt)
                    if m == 0:
                        b1sb = wpool.tile([128, M1], F32, tag="b1")
                        nc.sync.dma_start(
                            b1sb[:],
                            b1_d[g].rearrange("(m p) -> p m", p=128))
                        b2sb = wpool.tile([128, M1], F32, tag="b2")
                        nc.sync.dma_start(
                            b2sb[:],
                            b2_d[g].rearrange("(m p) -> p m", p=128))
                        b3sb = wpool.tile([C, 1], F32, tag="b3")
                        nc.sync.dma_start(b3sb[:], b3_d[g][:, None])
                for m in range(M1):
                    wt = wpool.tile([128, KO2, 128], BF16, tag=f"w2m{m}")
                    nc.sync.dma_start(wt[:], w2_d[g, :, m])
                    w2m.append(wt)
                w3sb = wpool.tile([128, KO2, C], BF16, tag="w3")
                nc.sync.dma_start(w3sb[:], w3_d[g])
                return dict(w1=w1m, w2=w2m, w3=w3sb,
                            b1=b1sb, b2=b2sb, b3=b3sb)

            def emit_x(b):
                # x blocks ride the scalar HWDGE ring, parallel to the
                # weight stream on sync.
                xsb = xpool.tile([128, KO1, blk], BF16, tag="x")
                nc.scalar.dma_start(xsb[:], x_d[b])
                return xsb

            def emit_L1(W, xsb):
                h1sb = h1pool.tile([128, KO2, blk], BF16, tag="h1")
                for m in range(M1):
                    ps = pspool.tile([128, blk], F32, tag="ps")
                    for k in range(KO1):
                        nc.tensor.matmul(
                            ps[:],
                            W["w1"][m][:, k, :],
                            xsb[:, k, :],
                            start=(k == 0), stop=(k == KO1 - 1))
                    nc.vector.tensor_scalar(
                        h1sb[:, m, :], ps[:], W["b1"][:, m:m + 1], 0.0,
                        mybir.AluOpType.add, mybir.AluOpType.max)
                return h1sb

            def emit_L23(b, W, h1sb):
                h2sb = h2pool.tile([128, KO2, blk], BF16, tag="h2")
                for m in range(M1):
                    ps = pspool.tile([128, blk], F32, tag="ps")
                    for k in range(KO2):
                        nc.tensor.matmul(
                            ps[:],
                            W["w2"][m][:, k, :],
                            h1sb[:, k, :],
                            start=(k == 0), stop=(k == KO2 - 1))
                    nc.scalar.activation(
                        h2sb[:, m, :], ps[:], relu, bias=W["b2"][:, m:m + 1])
                ps3 = pspool.tile([128, blk], F32, tag="ps")
                for k in range(KO2):
                    nc.tensor.matmul(
                        ps3[:C, :],
                        W["w3"][:, k, :],
                        h2sb[:, k, :],
                        start=(k == 0), stop=(k == KO2 - 1))
                osb = opool.tile([C, blk], F32, tag="o")
                nc.scalar.activation(
                    osb[:], ps3[:C, :], ident, bias=W["b3"][:, 0:1])
                nc.gpsimd.dma_start(out_d[b], osb[:])

            # Software pipeline, depth 2: L1 of blocks b+1/b+2 are
            # emitted before L2/L3 of block b, so weight-set DMAs and
            # ACT latency never drain the PE.
            Ws = {}
            h1 = {}

            def emit_front(b):
                g = runs[b]
                if g not in Ws:
                    Ws[g] = emit_weights(g)
                h1[b] = emit_L1(Ws[g], emit_x(b))

            # Startup: interleave the first NSTART blocks' L1 chains by
            # m-chunk so the in-order PE always has a ready chain while
            # W1 streams in (x rides the scalar ring in parallel).
            NSTART = min(3, NB)
            xs = {b: emit_x(b) for b in range(NSTART)}
            for b in range(NSTART):
                if runs[b] not in Ws:
                    Ws[runs[b]] = emit_weights(runs[b])
                h1[b] = h1pool.tile([128, KO2, blk], BF16, tag="h1",
                                    name=f"h1s{b}")
            for m in range(M1):
                for b in range(NSTART):
                    W = Ws[runs[b]]
                    ps = pspool.tile([128, blk], F32, tag="ps",
                                     name=f"ps_s{b}_{m}")
                    for k in range(KO1):
                        nc.tensor.matmul(
                            ps[:], W["w1"][m][:, k, :], xs[b][:, k, :],
                            start=(k == 0), stop=(k == KO1 - 1))
                    nc.vector.tensor_scalar(
                        h1[b][:, m, :], ps[:], W["b1"][:, m:m + 1], 0.0,
                        mybir.AluOpType.add, mybir.AluOpType.max)
            emitted = NSTART - 1
            for b in range(NB):
                for nxt in range(emitted + 1, min(b + 3, NB)):
                    emit_front(nxt)
                    emitted = nxt
                if b + 4 < NB and runs[b + 4] not in Ws:
                    Ws[runs[b + 4]] = emit_weights(runs[b + 4])
                emit_L23(b, Ws[runs[b]], h1.pop(b))

    nc.compile()
    _program_cache[key] = nc
    return nc


# ---------------------------------------------------------------- host
def _execute(inputs, trace=False, trace_cores=None):
    graph = np.ascontiguousarray(inputs["graph"], dtype=np.float32)
    state = np.ascontiguousarray(inputs["state"], dtype=np.float32)
    next_state = np.ascontiguousarray(inputs["next_state"], dtype=np.float32)
    W1 = np.ascontiguousarray(inputs["W1"], dtype=np.float32)
    b1 = np.ascontiguousarray(inputs["b1"], dtype=np.float32)
    W2 = np.ascontiguousarray(inputs["W2"], dtype=np.float32)
    b2 = np.ascontiguousarray(inputs["b2"], dtype=np.float32)
    W3 = np.ascontiguousarray(inputs["W3"], dtype=np.float32)
    b3 = np.ascontiguousarray(inputs["b3"], dtype=np.float32)

    B = graph.shape[0]
    NF, IN, H = W1.shape
    C = W3.shape[2]
    assert IN == graph.shape[1] + state.shape[1] + next_state.shape[1]
    assert H % 128 == 0 and C <= 128
    INP = ((IN + 127) // 128) * 128
    KO1 = INP // 128

    out_full = np.zeros((B, C), dtype=np.float32)

    # --- route: last active factor per row
    mask = graph[:, :NF] == 1.0
    active = mask.any(axis=1)
    last = (NF - 1) - np.argmax(mask[:, ::-1], axis=1)
    if not active.any():
        return (out_full, None) if trace else out_full

    rows_by_e = [np.nonzero(active & (last == e))[0] for e in range(NF)]
    nblk = [(len(r) + BLK - 1) // BLK for r in rows_by_e]
    prof, expert_of = _make_plan(nblk)
    G, NB = len(prof), sum(prof)

    # --- pack rows into per-core block slots
    # rowmap[core] : int32 [NB, BLK], original row id or -1 (pad)
    rowmap = [np.full((NB, BLK), -1, dtype=np.int64) for _ in range(NCORES)]
    off = np.cumsum([0] + prof)  # run g occupies blocks [off[g], off[g+1])
    slots_by_e = {}
    for core in range(NCORES):
        for g in range(G):
            slots_by_e.setdefault(expert_of[core][g], []).append((core, g))
    for e in range(NF):
        rows = rows_by_e[e]
        if len(rows) == 0:
            continue
        pos = 0
        for core, g in slots_by_e.get(e, []):
            cap = prof[g] * BLK
            take = min(cap, len(rows) - pos)
            if take <= 0:
                break
            flat = rowmap[core][off[g]:off[g + 1]].reshape(-1)
            flat[:take] = rows[pos:pos + take]
            pos += take
        assert pos == len(rows), f"expert {e} rows not fully packed"

    # --- build per-core inputs
    x = np.concatenate([graph, state, next_state], axis=1)  # [B, IN]
    if INP != IN:
        x = np.concatenate([x, np.zeros((B, INP - IN), np.float32)], axis=1)
    xpad = np.concatenate([x, np.zeros((1, INP), np.float32)], axis=0)
    W1p = np.zeros((NF, INP, H), np.float32)
    W1p[:, :IN] = W1

    # m-major device layouts: [.., 128, M1, KO, 128] so every m-chunk DMA
    # line is one contiguous ~5KB run per partition.
    KO2 = H // 128
    M1 = H // 128
    W1pm = np.ascontiguousarray(
        W1p.reshape(NF, KO1, 128, M1, 128).transpose(0, 2, 3, 1, 4)
    ).astype(NPBF16)
    W2pm = np.ascontiguousarray(
        W2.reshape(NF, KO2, 128, M1, 128).transpose(0, 2, 3, 1, 4)
    ).astype(NPBF16)
    W3pm = np.ascontiguousarray(
        W3.reshape(NF, KO2, 128, C).transpose(0, 2, 1, 3)).astype(NPBF16)
    xpad = xpad.astype(NPBF16)
    in_maps = []
    for core in range(NCORES):
        xb = xpad[rowmap[core].reshape(-1)]  # [NB*BLK, INP]; -1 -> zero row
        xb = np.ascontiguousarray(
            xb.reshape(NB, BLK, KO1, 128).transpose(0, 3, 2, 1))
        es = expert_of[core]
        in_maps.append({
            "xb": xb,
            "w1": W1pm[es],
            "w2": W2pm[es],
            "w3": W3pm[es],
            "b1": np.ascontiguousarray(b1[es]),
            "b2": np.ascontiguousarray(b2[es]),
            "b3": np.ascontiguousarray(b3[es]),
        })

    nc = _build_program(prof, KO1, KO2, H, C, BLK)
    kwargs = {}
    if trace:
        kwargs = dict(trace=True,
                      trace_cores=trace_cores or list(range(NCORES)))
    res = run_bass_kernel_spmd(nc, in_maps, list(range(NCORES)), **kwargs)

    # --- scatter back
    for core in range(NCORES):
        ob = np.asarray(res.results[core]["outb"])  # [NB, C, BLK]
        rows = ob.transpose(0, 2, 1).reshape(NB * BLK, C)
        ids = rowmap[core].reshape(-1)
        valid = ids >= 0
        out_full[ids[valid]] = rows[valid]

    return (out_full, res) if trace else out_full


def kernel(**inputs):
    return _execute(inputs)


# revision 17
# speedup vs baseline: 1.1689x; 1.1689x over previous
"""MoE-routed DIAYN discriminator kernel for 8 Trainium2 NeuronCores.

Reference semantics: x = concat([graph, state, next_state], -1); for each
row, run the 3-layer MLP of the LAST factor i<NF with graph[:, i]==1
(rows with no active factor output 0). The dense reference computes all
NF expert MLPs for every row; we instead route each row to exactly one
expert on the host, pack rows into 8 SPMD shards, and run one dense
per-expert MLP stream per core.

Sharding: rows are grouped by expert into BLK-row blocks. Every core
executes the same static "profile" of G runs (run g = prof[g] blocks);
each run uses one weight set, supplied per-core as data. A small host-side
search picks (G, prof) and an assignment of runs -> experts that covers
the actual per-expert block counts with minimal padding + weight traffic.

Device kernel (per run, per block, activations kept transposed [feat, row]):
  h1 = relu(W1^T x + b1); h2 = relu(W2^T h1 + b2); out = W3^T h2 + b3
matmuls run in bf16 (same 1 row/cycle PE rate as fp32r but ~10ns less
per-MM overhead, LDWEIGHTS fully hidden, and half the DMA bytes); PSUM
accumulation, biases and the output stay fp32 (rel err ~4.5e-3 vs the
fp32 reference, well inside the 2e-2 gate).
Weights stream m-major ([128, KO, 128] chunk per output m-tile) so each
arriving chunk enables complete k-accumulation chains; the first three
blocks' chains are emitted m-interleaved so the in-order PE always has
runnable work while W1 streams in (x rides the scalar ring in parallel).
"""

import numpy as np

import concourse.bass as bass
import concourse.mybir as mybir
from concourse import bacc
from concourse.tile import TileContext
from concourse.bass_utils import run_bass_kernel_spmd

NCORES = 8
BLK = 272  # rows per matmul block; >=256 (f32r full rate), <=512 (PSUM bank)

F32 = mybir.dt.float32
BF16 = mybir.dt.bfloat16
NPBF16 = mybir.dt.np(BF16)

# Rough per-core cost weights for the plan search (ns).
_COST_BLOCK = int(152 * (BLK / 2.4 + 3))  # PE ns per block (152 matmuls)
_COST_RUN = 12_000  # partially-exposed weight-set DMA per extra run

_program_cache = {}


# ---------------------------------------------------------------- planning
def _compositions(total, parts):
    """Non-increasing positive integer compositions of `total` into `parts`."""
    if parts == 1:
        yield (total,)
        return
    for first in range((total + parts - 1) // parts, total - parts + 2):
        for rest in _compositions(total - first, parts - 1):
            if rest[0] <= first:
                yield (first,) + rest


def _try_assign(demands, prof):
    """Greedy cover of per-expert block demands by the 8x-replicated profile.

    demands: list of (n_blocks, expert) sorted desc. Returns dict
    run_size -> list of experts (8 entries per profile slot of that size,
    padding slots filled with the largest expert) or None if infeasible.
    """
    runs = sorted([t for t in prof for _ in range(NCORES)], reverse=True)
    used = []  # (size, expert)
    for n, e in demands:
        rem = n
        while rem > 0:
            if not runs:
                return None
            # largest run <= rem, else smallest run (minimal overshoot)
            pick = None
            for i, s in enumerate(runs):
                if s <= rem:
                    pick = i
                    break
            if pick is None:
                pick = len(runs) - 1
            s = runs.pop(pick)
            used.append((s, e))
            rem -= s
    pad_expert = demands[0][1]
    for s in runs:
        used.append((s, pad_expert))
    by_size = {}
    for s, e in used:
        by_size.setdefault(s, []).append(e)
    return by_size


def _make_plan(nblk):
    """nblk: per-expert block counts. Returns (prof, expert_of[core][g])."""
    demands = sorted(
        [(n, e) for e, n in enumerate(nblk) if n > 0], reverse=True
    )
    total = sum(n for n, _ in demands)
    mincap = (total + NCORES - 1) // NCORES
    best = None
    for G in range(1, 9):
        for cap in range(mincap, mincap + 6):
            for prof in _compositions(cap, G):
                a = _try_assign(demands, prof)
                if a is None:
                    continue
                cost = cap * _COST_BLOCK + G * _COST_RUN
                if best is None or cost < best[0]:
                    best = (cost, prof, a)
    assert best is not None, "no feasible run plan found"
    _, prof, by_size = best
    queues = {s: list(es) for s, es in by_size.items()}
    expert_of = [[None] * len(prof) for _ in range(NCORES)]
    for g, s in enumerate(prof):
        for core in range(NCORES):
            expert_of[core][g] = queues[s].pop(0)
    return list(prof), expert_of


# ---------------------------------------------------------------- device
def _build_program(prof, KO1, KO2, H, C, blk):
    """Build + compile the SPMD Bass program for a run profile."""
    key = (tuple(prof), KO1, KO2, H, C, blk)
    if key in _program_cache:
        return _program_cache[key]

    G = len(prof)
    NB = sum(prof)
    M1 = H // 128
    relu = mybir.ActivationFunctionType.Relu
    ident = mybir.ActivationFunctionType.Identity

    nc = bacc.Bacc("TRN2", target_bir_lowering=False, debug=False,
                   num_devices=NCORES)
    x_d = nc.dram_tensor("xb", [NB, 128, KO1, blk], BF16,
                         kind="ExternalInput").ap()
    # m-major weight layout: [G, 128, M1, KO, 128]
    w1_d = nc.dram_tensor("w1", [G, 128, M1, KO1, 128], BF16,
                          kind="ExternalInput").ap()
    w2_d = nc.dram_tensor("w2", [G, 128, M1, KO2, 128], BF16,
                          kind="ExternalInput").ap()
    w3_d = nc.dram_tensor("w3", [G, 128, KO2, C], BF16,
                          kind="ExternalInput").ap()
    b1_d = nc.dram_tensor("b1", [G, H], F32, kind="ExternalInput").ap()
    b2_d = nc.dram_tensor("b2", [G, H], F32, kind="ExternalInput").ap()
    b3_d = nc.dram_tensor("b3", [G, C], F32, kind="ExternalInput").ap()
    out_d = nc.dram_tensor("outb", [NB, C, blk], F32, kind="ExternalOutput").ap()

    runs = []
    for g, T in enumerate(prof):
        runs += [g] * T

    with TileContext(nc) as tc:
        with (
            tc.tile_pool(name="w", bufs=2) as wpool,
            tc.tile_pool(name="x", bufs=3) as xpool,
            tc.tile_pool(name="h1", bufs=4) as h1pool,
            tc.tile_pool(name="h2", bufs=1) as h2pool,
            tc.tile_pool(name="o", bufs=2) as opool,
            tc.tile_pool(name="ps", bufs=8, space="PSUM") as pspool,
        ):
            def emit_weights(g):
                # m-major stream on the sync ring: W1 m-chunks (each one
                # enables a full k-chain), biases early, then W2 m-chunks,
                # then W3.
                w1m, w2m = [], []
                for m in range(M1):
                    wt = wpool.tile([128, KO1, 128], BF16, tag=f"w1m{m}")
                    nc.sync.dma_start(wt[:], w1_d[g, :, m])
                    w1m.append(wt)
                    if m == 0:
                        b1sb = wpool.tile([128, M1], F32, tag="b1")
                        nc.sync.dma_start(
                            b1sb[:],
                            b1_d[g].rearrange("(m p) -> p m", p=128))
                        b2sb = wpool.tile([128, M1], F32, tag="b2")
                        nc.sync.dma_start(
                            b2sb[:],
                            b2_d[g].rearrange("(m p) -> p m", p=128))
                        b3sb = wpool.tile([C, 1], F32, tag="b3")
                        nc.sync.dma_start(b3sb[:], b3_d[g][:, None])
                for m in range(M1):
                    wt = wpool.tile([128, KO2, 128], BF16, tag=f"w2m{m}")
                    nc.sync.dma_start(wt[:], w2_d[g, :, m])
                    w2m.append(wt)
                w3sb = wpool.tile([128, KO2, C], BF16, tag="w3")
                nc.sync.dma_start(w3sb[:], w3_d[g])
                return dict(w1=w1m, w2=w2m, w3=w3sb,
                            b1=b1sb, b2=b2sb, b3=b3sb)

            def emit_x(b):
                # x blocks ride the scalar HWDGE ring, parallel to the
                # weight stream on sync.
                xsb = xpool.tile([128, KO1, blk], BF16, tag="x")
                nc.scalar.dma_start(xsb[:], x_d[b])
                return xsb

            def emit_L1(W, xsb):
                h1sb = h1pool.tile([128, KO2, blk], BF16, tag="h1")
                for m in range(M1):
                    ps = pspool.tile([128, blk], F32, tag="ps")
                    for k in range(KO1):
                        nc.tensor.matmul(
                            ps[:],
                            W["w1"][m][:, k, :],
                            xsb[:, k, :],
                            start=(k == 0), stop=(k == KO1 - 1))
                    nc.vector.tensor_scalar(
                        h1sb[:, m, :], ps[:], W["b1"][:, m:m + 1], 0.0,
                        mybir.AluOpType.add, mybir.AluOpType.max)
                return h1sb

            def emit_L23(b, W, h1sb):
                h2sb = h2pool.tile([128, KO2, blk], BF16, tag="h2")
                for m in range(M1):
                    ps = pspool.tile([128, blk], F32, tag="ps")
                    for k in range(KO2):
                        nc.tensor.matmul(
                            ps[:],
                            W["w2"][m][:, k, :],
                            h1sb[:, k, :],
                            start=(k == 0), stop=(k == KO2 - 1))
                    nc.scalar.activation(
                        h2sb[:, m, :], ps[:], relu, bias=W["b2"][:, m:m + 1])
                ps3 = pspool.tile([128, blk], F32, tag="ps")
                for k in range(KO2):
                    nc.tensor.matmul(
                        ps3[:C, :],
                        W["w3"][:, k, :],
                        h2sb[:, k, :],
                        start=(k == 0), stop=(k == KO2 - 1))
                osb = opool.tile([C, blk], F32, tag="o")
                nc.scalar.activation(
                    osb[:], ps3[:C, :], ident, bias=W["b3"][:, 0:1])
                if b >= NB - 2:
                    # last blocks ride the scalar HWDGE ring so the slow
                    # gpsimd SWDGE drain isn't serialized at kernel end
                    nc.scalar.dma_start(out_d[b], osb[:])
                else:
                    nc.gpsimd.dma_start(out_d[b], osb[:])

            # Software pipeline, depth 2: L1 of blocks b+1/b+2 are
            # emitted before L2/L3 of block b, so weight-set DMAs and
            # ACT latency never drain the PE.
            Ws = {}
            h1 = {}

            def emit_front(b):
                g = runs[b]
                if g not in Ws:
                    Ws[g] = emit_weights(g)
                h1[b] = emit_L1(Ws[g], emit_x(b))

            # Startup: interleave the first NSTART blocks' L1 chains by
            # m-chunk so the in-order PE always has a ready chain while
            # W1 streams in (x rides the scalar ring in parallel).
            NSTART = min(3, NB)
            xs = {b: emit_x(b) for b in range(NSTART)}
            for b in range(NSTART):
                if runs[b] not in Ws:
                    Ws[runs[b]] = emit_weights(runs[b])
                h1[b] = h1pool.tile([128, KO2, blk], BF16, tag="h1",
                                    name=f"h1s{b}")
            for m in range(M1):
                for b in range(NSTART):
                    W = Ws[runs[b]]
                    ps = pspool.tile([128, blk], F32, tag="ps",
                                     name=f"ps_s{b}_{m}")
                    for k in range(KO1):
                        nc.tensor.matmul(
                            ps[:], W["w1"][m][:, k, :], xs[b][:, k, :],
                            start=(k == 0), stop=(k == KO1 - 1))
                    nc.vector.tensor_scalar(
                        h1[b][:, m, :], ps[:], W["b1"][:, m:m + 1], 0.0,
                        mybir.AluOpType.add, mybir.AluOpType.max)
            emitted = NSTART - 1
            for b in range(NB):
                for nxt in range(emitted + 1, min(b + 3, NB)):
                    emit_front(nxt)
                    emitted = nxt
                if b + 4 < NB and runs[b + 4] not in Ws:
                    Ws[runs[b + 4]] = emit_weights(runs[b + 4])
                emit_L23(b, Ws[runs[b]], h1.pop(b))

    nc.compile()
    _program_cache[key] = nc
    return nc


# ---------------------------------------------------------------- host
def _execute(inputs, trace=False, trace_cores=None):
    graph = np.ascontiguousarray(inputs["graph"], dtype=np.float32)
    state = np.ascontiguousarray(inputs["state"], dtype=np.float32)
    next_state = np.ascontiguousarray(inputs["next_state"], dtype=np.float32)
    W1 = np.ascontiguousarray(inputs["W1"], dtype=np.float32)
    b1 = np.ascontiguousarray(inputs["b1"], dtype=np.float32)
    W2 = np.ascontiguousarray(inputs["W2"], dtype=np.float32)
    b2 = np.ascontiguousarray(inputs["b2"], dtype=np.float32)
    W3 = np.ascontiguousarray(inputs["W3"], dtype=np.float32)
    b3 = np.ascontiguousarray(inputs["b3"], dtype=np.float32)

    B = graph.shape[0]
    NF, IN, H = W1.shape
    C = W3.shape[2]
    assert IN == graph.shape[1] + state.shape[1] + next_state.shape[1]
    assert H % 128 == 0 and C <= 128
    INP = ((IN + 127) // 128) * 128
    KO1 = INP // 128

    out_full = np.zeros((B, C), dtype=np.float32)

    # --- route: last active factor per row
    mask = graph[:, :NF] == 1.0
    active = mask.any(axis=1)
    last = (NF - 1) - np.argmax(mask[:, ::-1], axis=1)
    if not active.any():
        return (out_full, None) if trace else out_full

    rows_by_e = [np.nonzero(active & (last == e))[0] for e in range(NF)]
    nblk = [(len(r) + BLK - 1) // BLK for r in rows_by_e]
    prof, expert_of = _make_plan(nblk)
    G, NB = len(prof), sum(prof)

    # --- pack rows into per-core block slots
    # rowmap[core] : int32 [NB, BLK], original row id or -1 (pad)
    rowmap = [np.full((NB, BLK), -1, dtype=np.int64) for _ in range(NCORES)]
    off = np.cumsum([0] + prof)  # run g occupies blocks [off[g], off[g+1])
    slots_by_e = {}
    for core in range(NCORES):
        for g in range(G):
            slots_by_e.setdefault(expert_of[core][g], []).append((core, g))
    for e in range(NF):
        rows = rows_by_e[e]
        if len(rows) == 0:
            continue
        pos = 0
        for core, g in slots_by_e.get(e, []):
            cap = prof[g] * BLK
            take = min(cap, len(rows) - pos)
            if take <= 0:
                break
            flat = rowmap[core][off[g]:off[g + 1]].reshape(-1)
            flat[:take] = rows[pos:pos + take]
            pos += take
        assert pos == len(rows), f"expert {e} rows not fully packed"

    # --- build per-core inputs
    x = np.concatenate([graph, state, next_state], axis=1)  # [B, IN]
    if INP != IN:
        x = np.concatenate([x, np.zeros((B, INP - IN), np.float32)], axis=1)
    xpad = np.concatenate([x, np.zeros((1, INP), np.float32)], axis=0)
    W1p = np.zeros((NF, INP, H), np.float32)
    W1p[:, :IN] = W1

    # m-major device layouts: [.., 128, M1, KO, 128] so every m-chunk DMA
    # line is one contiguous ~5KB run per partition.
    KO2 = H // 128
    M1 = H // 128
    W1pm = np.ascontiguousarray(
        W1p.reshape(NF, KO1, 128, M1, 128).transpose(0, 2, 3, 1, 4)
    ).astype(NPBF16)
    W2pm = np.ascontiguousarray(
        W2.reshape(NF, KO2, 128, M1, 128).transpose(0, 2, 3, 1, 4)
    ).astype(NPBF16)
    W3pm = np.ascontiguousarray(
        W3.reshape(NF, KO2, 128, C).transpose(0, 2, 1, 3)).astype(NPBF16)
    xpad = xpad.astype(NPBF16)
    in_maps = []
    for core in range(NCORES):
        xb = xpad[rowmap[core].reshape(-1)]  # [NB*BLK, INP]; -1 -> zero row
        xb = np.ascontiguousarray(
            xb.reshape(NB, BLK, KO1, 128).transpose(0, 3, 2, 1))
        es = expert_of[core]
        in_maps.append({
            "xb": xb,
            "w1": W1pm[es],
            "w2": W2pm[es],
            "w3": W3pm[es],
            "b1": np.ascontiguousarray(b1[es]),
            "b2": np.ascontiguousarray(b2[es]),
            "b3": np.ascontiguousarray(b3[es]),
        })

    nc = _build_program(prof, KO1, KO2, H, C, BLK)
    kwargs = {}
    if trace:
        kwargs = dict(trace=True,
                      trace_cores=trace_cores or list(range(NCORES)))
    res = run_bass_kernel_spmd(nc, in_maps, list(range(NCORES)), **kwargs)

    # --- scatter back
    for core in range(NCORES):
        ob = np.asarray(res.results[core]["outb"])  # [NB, C, BLK]
        rows = ob.transpose(0, 2, 1).reshape(NB * BLK, C)
        ids = rowmap[core].reshape(-1)
        valid = ids >= 0
        out_full[ids[valid]] = rows[valid]

    return (out_full, res) if trace else out_full


def kernel(**inputs):
    return _execute(inputs)


# revision 19
# speedup vs baseline: 1.2197x; 1.0435x over previous
"""MoE-routed DIAYN discriminator kernel for 8 Trainium2 NeuronCores.

Reference semantics: x = concat([graph, state, next_state], -1); for each
row, run the 3-layer MLP of the LAST factor i<NF with graph[:, i]==1
(rows with no active factor output 0). The dense reference computes all
NF expert MLPs for every row; we instead route each row to exactly one
expert on the host, pack rows into 8 SPMD shards, and run one dense
per-expert MLP stream per core.

Sharding: rows are grouped by expert into blocks. Every core executes
the same static profile of G runs (run g = T_g blocks of blk_g rows);
each run uses one weight set, supplied per-core as data. A host-side
search picks the profile (up to two distinct block sizes, e.g. seven
272-row blocks plus one 192-row tail block) and an expert-to-slot
assignment covering the per-expert row counts with minimal PE time.

Device kernel (per run, per block, activations kept transposed [feat, row]):
  h1 = relu(W1^T x + b1); h2 = relu(W2^T h1 + b2); out = W3^T h2 + b3
matmuls run in bf16 (1 row/cycle like fp32r but ~10ns less per-MM
overhead, LDWEIGHTS fully hidden, half the DMA bytes); PSUM
accumulation, biases and the output stay fp32 (rel err ~4.5e-3, gate
2e-2). Weights stream m-major so each arriving chunk enables complete
k-chains; the first blocks' chains are emitted m-interleaved so the
in-order PE always has runnable work while W1 streams (x rides the
scalar ring in parallel). The last blocks' output DMAs use the scalar
HWDGE ring so the gpsimd SWDGE drain is not serialized at kernel end.
"""

import numpy as np

import concourse.mybir as mybir
from concourse import bacc
from concourse.tile import TileContext
from concourse.bass_utils import run_bass_kernel_spmd

NCORES = 8
BLK = 272  # fallback uniform block size
_NMM = 152  # matmuls per block (80 L1 + 64 L2 + 8 L3)

F32 = mybir.dt.float32
BF16 = mybir.dt.bfloat16
NPBF16 = mybir.dt.np(BF16)

_program_cache = {}


# ---------------------------------------------------------------- planning
def _t_mm(n):
    """bf16 LDW+MM pair pacing (measured on hw)."""
    return max(56.0, n / 2.4 + 2.5)


def _feasible(demands, slot_caps, node_cap=200000):
    """demands: desc row counts. slot_caps: list of (cap, count).
    Returns per-demand counts per cap-type (types desc) or None."""
    types = sorted(slot_caps, reverse=True)
    caps = [t[0] for t in types]
    counts = [t[1] for t in types]
    nodes = [0]

    def dfs(ei, avail):
        if nodes[0] > node_cap:
            return None
        nodes[0] += 1
        if ei == len(demands):
            return []
        best = None

        def alloc(ti, rem, cur, avail2):
            nonlocal best
            if best is not None or nodes[0] > node_cap:
                return
            nodes[0] += 1
            if rem <= 0:
                rest = dfs(ei + 1, avail2)
                if rest is not None:
                    best = [list(cur)] + rest
                return
            if ti == len(caps):
                return
            maxn = min(avail2[ti], (rem + caps[ti] - 1) // caps[ti])
            for n in range(maxn, -1, -1):
                cur.append(n)
                a3 = list(avail2)
                a3[ti] -= n
                alloc(ti + 1, rem - n * caps[ti], cur, a3)
                cur.pop()
                if best is not None:
                    return

        alloc(0, demands[ei], [], list(avail))
        return best

    res = dfs(0, counts)
    return res, types


def _search_profile(rows, run_cost=3000.0,
                    blks=(512, 448, 384, 320, 272, 224, 192, 160, 136),
                    maxG=4, maxT=8, cap_slack=1.12, max_sizes=2):
    """Best profile [(T, blk), ...] + per-expert slot counts, or None."""
    order = sorted(range(len(rows)), key=lambda e: -rows[e])
    demands = [rows[e] for e in order if rows[e] > 0]
    order = [e for e in order if rows[e] > 0]
    ideal = -(-sum(demands) // NCORES)
    types = [(T, blk, T * blk, T * _NMM * _t_mm(blk))
             for blk in blks for T in range(1, maxT + 1)]
    cands = []
    limit = ideal * cap_slack

    def rec(start, chosen, cap, cost, sizes):
        if cap >= ideal:
            cands.append((cost + len(chosen) * run_cost, cap, list(chosen)))
        if len(chosen) == maxG or cap >= limit:
            return
        for i in range(start, len(types)):
            T, blk, tcap, tcost = types[i]
            ns = sizes | {blk}
            if len(ns) > max_sizes or cap + tcap > limit:
                continue
            chosen.append(i)
            rec(i, chosen, cap + tcap, cost + tcost, ns)
            chosen.pop()

    rec(0, [], 0, 0.0, set())
    cands.sort(key=lambda c: c[0])
    for cost, cap, chosen in cands[:4000]:
        slot_caps = {}
        for i in chosen:
            T, blk, tcap, _ = types[i]
            slot_caps[tcap] = slot_caps.get(tcap, 0) + NCORES
        res, typ = _feasible(demands, list(slot_caps.items()))
        if res is not None:
            prof = sorted([(types[i][0], types[i][1]) for i in chosen],
                          key=lambda tb: (-tb[0] * tb[1], -tb[1]))
            return prof, order, res, typ
    return None


def _make_plan(rows):
    """rows: per-expert row counts. Returns (prof [(T, blk)...],
    expert_of[core][g])."""
    found = _search_profile(rows)
    if found is None:
        found = _search_profile(rows, maxG=6, maxT=10, cap_slack=1.6,
                                max_sizes=2)
    if found is None:
        # fallback: uniform blocks, simple greedy profile
        nblk = [(r + BLK - 1) // BLK for r in rows]
        total = sum(nblk)
        cap = -(-total // NCORES)
        prof = [(cap, BLK)]
        demands = sorted([(n, e) for e, n in enumerate(nblk) if n > 0],
                         reverse=True)
        # one run per core of cap blocks can always cover by splitting
        # experts across slots greedily
        slots = []
        for n, e in demands:
            rem = n
            while rem > 0:
                slots.append(e)
                rem -= cap
        slots += [demands[0][1]] * (NCORES - len(slots))
        assert len(slots) <= NCORES, "fallback plan infeasible"
        return prof, [[slots[c]] for c in range(NCORES)]

    prof, order, res, typ = found
    # queues of experts per slot-capacity
    queues = {}
    for ei, counts in enumerate(res):
        e = order[ei]
        for ti, n in enumerate(counts):
            cap = typ[ti][0]
            queues.setdefault(cap, []).extend([e] * n)
    pad_e = order[0]
    expert_of = [[None] * len(prof) for _ in range(NCORES)]
    for g, (T, blk) in enumerate(prof):
        cap = T * blk
        q = queues.setdefault(cap, [])
        for core in range(NCORES):
            expert_of[core][g] = q.pop(0) if q else pad_e
    return prof, expert_of


# ---------------------------------------------------------------- device
def _build_program(prof, KO1, KO2, H, C):
    """Build + compile the SPMD Bass program for a run profile."""
    key = (tuple(prof), KO1, KO2, H, C)
    if key in _program_cache:
        return _program_cache[key]

    G = len(prof)
    NB = sum(T for T, _ in prof)
    M1 = H // 128
    relu = mybir.ActivationFunctionType.Relu
    ident = mybir.ActivationFunctionType.Identity

    # block table: global index -> (run, blk, size-class, class index)
    sizes = []
    for T, blk in prof:
        if blk not in sizes:
            sizes.append(blk)
    runs, blkof, classof, cidxof = [], [], [], []
    ccount = {s: 0 for s in sizes}
    for g, (T, blk) in enumerate(prof):
        for _ in range(T):
            runs.append(g)
            blkof.append(blk)
            classof.append(sizes.index(blk))
            cidxof.append(ccount[blk])
            ccount[blk] += 1

    nc = bacc.Bacc("TRN2", target_bir_lowering=False, debug=False,
                   num_devices=NCORES)
    x_ds, out_ds = [], []
    for si, s in enumerate(sizes):
        x_ds.append(nc.dram_tensor(f"xb{si}", [ccount[s], 128, KO1, s],
                                   BF16, kind="ExternalInput").ap())
        out_ds.append(nc.dram_tensor(f"outb{si}", [ccount[s], C, s],
                                     F32, kind="ExternalOutput").ap())
    # m-major weight layout: [G, 128, M1, KO, 128]
    w1_d = nc.dram_tensor("w1", [G, 128, M1, KO1, 128], BF16,
                          kind="ExternalInput").ap()
    w2_d = nc.dram_tensor("w2", [G, 128, M1, KO2, 128], BF16,
                          kind="ExternalInput").ap()
    w3_d = nc.dram_tensor("w3", [G, 128, KO2, C], BF16,
                          kind="ExternalInput").ap()
    b1_d = nc.dram_tensor("b1", [G, H], F32, kind="ExternalInput").ap()
    b2_d = nc.dram_tensor("b2", [G, H], F32, kind="ExternalInput").ap()
    b3_d = nc.dram_tensor("b3", [G, C], F32, kind="ExternalInput").ap()

    with TileContext(nc) as tc:
        with (
            tc.tile_pool(name="w", bufs=2) as wpool,
            tc.tile_pool(name="x", bufs=3) as xpool,
            tc.tile_pool(name="h1", bufs=4) as h1pool,
            tc.tile_pool(name="h2", bufs=1) as h2pool,
            tc.tile_pool(name="o", bufs=2) as opool,
            tc.tile_pool(name="ps", bufs=8, space="PSUM") as pspool,
        ):
            def emit_weights(g):
                # m-major stream on the sync ring: W1 m-chunks (each
                # enables a full k-chain), biases early, then W2, W3.
                w1m, w2m = [], []
                b1sb = b2sb = b3sb = None
                for m in range(M1):
                    wt = wpool.tile([128, KO1, 128], BF16, tag=f"w1m{m}")
                    nc.sync.dma_start(wt[:], w1_d[g, :, m])
                    w1m.append(wt)
                    if m == 0:
                        b1sb = wpool.tile([128, M1], F32, tag="b1")
                        nc.sync.dma_start(
                            b1sb[:],
                            b1_d[g].rearrange("(m p) -> p m", p=128))
                        b2sb = wpool.tile([128, M1], F32, tag="b2")
                        nc.sync.dma_start(
                            b2sb[:],
                            b2_d[g].rearrange("(m p) -> p m", p=128))
                        b3sb = wpool.tile([C, 1], F32, tag="b3")
                        nc.sync.dma_start(b3sb[:], b3_d[g][:, None])
                for m in range(M1):
                    wt = wpool.tile([128, KO2, 128], BF16, tag=f"w2m{m}")
                    nc.sync.dma_start(wt[:], w2_d[g, :, m])
                    w2m.append(wt)
                w3sb = wpool.tile([128, KO2, C], BF16, tag="w3")
                nc.sync.dma_start(w3sb[:], w3_d[g])
                return dict(w1=w1m, w2=w2m, w3=w3sb,
                            b1=b1sb, b2=b2sb, b3=b3sb)

            def emit_x(b):
                s = blkof[b]
                xsb = xpool.tile([128, KO1, s], BF16, tag=f"x{s}",
                                 name=f"x{b}")
                nc.scalar.dma_start(xsb[:], x_ds[classof[b]][cidxof[b]])
                return xsb

            def emit_L1_chain(W, xsb, h1sb, m, s):
                ps = pspool.tile([128, 512], F32, tag="ps")
                for k in range(KO1):
                    nc.tensor.matmul(
                        ps[:, :s], W["w1"][m][:, k, :], xsb[:, k, :],
                        start=(k == 0), stop=(k == KO1 - 1))
                nc.vector.tensor_scalar(
                    h1sb[:, m, :], ps[:, :s], W["b1"][:, m:m + 1], 0.0,
                    mybir.AluOpType.add, mybir.AluOpType.max)

            def emit_L1(b, W, xsb):
                s = blkof[b]
                h1sb = h1pool.tile([128, KO2, s], BF16, tag=f"h1{s}",
                                   name=f"h1_{b}")
                for m in range(M1):
                    emit_L1_chain(W, xsb, h1sb, m, s)
                return h1sb

            def emit_L23(b, W, h1sb):
                s = blkof[b]
                h2sb = h2pool.tile([128, KO2, s], BF16, tag=f"h2{s}",
                                   name=f"h2_{b}")
                for m in range(M1):
                    ps = pspool.tile([128, 512], F32, tag="ps")
                    for k in range(KO2):
                        nc.tensor.matmul(
                            ps[:, :s], W["w2"][m][:, k, :], h1sb[:, k, :],
                            start=(k == 0), stop=(k == KO2 - 1))
                    nc.scalar.activation(
                        h2sb[:, m, :], ps[:, :s], relu,
                        bias=W["b2"][:, m:m + 1])
                ps3 = pspool.tile([128, 512], F32, tag="ps")
                for k in range(KO2):
                    nc.tensor.matmul(
                        ps3[:C, :s], W["w3"][:, k, :], h2sb[:, k, :],
                        start=(k == 0), stop=(k == KO2 - 1))
                osb = opool.tile([C, s], F32, tag=f"o{s}", name=f"o{b}")
                nc.scalar.activation(
                    osb[:], ps3[:C, :s], ident, bias=W["b3"][:, 0:1])
                dst = out_ds[classof[b]][cidxof[b]]
                if b >= NB - 2:
                    # last blocks ride the scalar HWDGE ring so the slow
                    # gpsimd SWDGE drain isn't serialized at kernel end
                    nc.scalar.dma_start(dst, osb[:])
                else:
                    nc.gpsimd.dma_start(dst, osb[:])

            # Software pipeline, depth 2: L1 of blocks b+1/b+2 emitted
            # before L2/L3 of block b.
            Ws = {}
            h1 = {}

            def emit_front(b):
                g = runs[b]
                if g not in Ws:
                    Ws[g] = emit_weights(g)
                h1[b] = emit_L1(b, Ws[g], emit_x(b))

            # Startup: interleave the first NSTART blocks' L1 chains by
            # m-chunk so the in-order PE always has a ready chain while
            # W1 streams in (x rides the scalar ring in parallel).
            NSTART = min(3, NB)
            xs = {b: emit_x(b) for b in range(NSTART)}
            for b in range(NSTART):
                if runs[b] not in Ws:
                    Ws[runs[b]] = emit_weights(runs[b])
                s = blkof[b]
                h1[b] = h1pool.tile([128, KO2, s], BF16, tag=f"h1{s}",
                                    name=f"h1s{b}")
            for m in range(M1):
                for b in range(NSTART):
                    emit_L1_chain(Ws[runs[b]], xs[b], h1[b], m, blkof[b])
            emitted = NSTART - 1
            for b in range(NB):
                for nxt in range(emitted + 1, min(b + 3, NB)):
                    emit_front(nxt)
                    emitted = nxt
                if b + 4 < NB and runs[b + 4] not in Ws:
                    Ws[runs[b + 4]] = emit_weights(runs[b + 4])
                emit_L23(b, Ws[runs[b]], h1.pop(b))

    nc.compile()
    _program_cache[key] = nc
    return nc


# ---------------------------------------------------------------- host
def _execute(inputs, trace=False, trace_cores=None):
    graph = np.ascontiguousarray(inputs["graph"], dtype=np.float32)
    state = np.ascontiguousarray(inputs["state"], dtype=np.float32)
    next_state = np.ascontiguousarray(inputs["next_state"], dtype=np.float32)
    W1 = np.ascontiguousarray(inputs["W1"], dtype=np.float32)
    b1 = np.ascontiguousarray(inputs["b1"], dtype=np.float32)
    W2 = np.ascontiguousarray(inputs["W2"], dtype=np.float32)
    b2 = np.ascontiguousarray(inputs["b2"], dtype=np.float32)
    W3 = np.ascontiguousarray(inputs["W3"], dtype=np.float32)
    b3 = np.ascontiguousarray(inputs["b3"], dtype=np.float32)

    B = graph.shape[0]
    NF, IN, H = W1.shape
    C = W3.shape[2]
    assert IN == graph.shape[1] + state.shape[1] + next_state.shape[1]
    assert H % 128 == 0 and C <= 128
    INP = ((IN + 127) // 128) * 128
    KO1 = INP // 128

    out_full = np.zeros((B, C), dtype=np.float32)

    # --- route: last active factor per row
    mask = graph[:, :NF] == 1.0
    active = mask.any(axis=1)
    last = (NF - 1) - np.argmax(mask[:, ::-1], axis=1)
    if not active.any():
        return (out_full, None) if trace else out_full

    rows_by_e = [np.nonzero(active & (last == e))[0] for e in range(NF)]
    prof, expert_of = _make_plan([len(r) for r in rows_by_e])
    G, NB = len(prof), sum(T for T, _ in prof)

    # block table (must match _build_program)
    sizes = []
    for T, blk in prof:
        if blk not in sizes:
            sizes.append(blk)
    runs, blkof, classof, cidxof = [], [], [], []
    ccount = {s: 0 for s in sizes}
    roff = []  # per-run start block index
    for g, (T, blk) in enumerate(prof):
        roff.append(len(runs))
        for _ in range(T):
            runs.append(g)
            blkof.append(blk)
            classof.append(sizes.index(blk))
            cidxof.append(ccount[blk])
            ccount[blk] += 1

    # --- pack rows into per-core, per-class block slots
    # rowmap[core][class]: [n_class_blocks, s] original row id or -1
    rowmap = [[np.full((ccount[s], s), -1, dtype=np.int64) for s in sizes]
              for _ in range(NCORES)]
    slots_by_e = {}
    for core in range(NCORES):
        for g in range(G):
            slots_by_e.setdefault(expert_of[core][g], []).append((core, g))
    for e in range(NF):
        rows = rows_by_e[e]
        if len(rows) == 0:
            continue
        pos = 0
        for core, g in slots_by_e.get(e, []):
            T, blk = prof[g]
            cap = T * blk
            take = min(cap, len(rows) - pos)
            if take <= 0:
                break
            si = sizes.index(blk)
            c0 = cidxof[roff[g]]
            flat = rowmap[core][si][c0:c0 + T].reshape(-1)
            flat[:take] = rows[pos:pos + take]
            pos += take
        assert pos == len(rows), f"expert {e} rows not fully packed"

    # --- build per-core inputs
    x = np.concatenate([graph, state, next_state], axis=1)  # [B, IN]
    if INP != IN:
        x = np.concatenate([x, np.zeros((B, INP - IN), np.float32)], axis=1)
    xpad = np.concatenate([x, np.zeros((1, INP), np.float32)], axis=0)
    xpad = xpad.astype(NPBF16)
    W1p = np.zeros((NF, INP, H), NPBF16)
    W1p[:, :IN] = W1.astype(NPBF16)

    # m-major device layouts: [.., 128, M1, KO, 128] so every m-chunk DMA
    # line is one contiguous run per partition.
    KO2 = H // 128
    M1 = H // 128
    W1pm = np.ascontiguousarray(
        W1p.reshape(NF, KO1, 128, M1, 128).transpose(0, 2, 3, 1, 4))
    W2pm = np.ascontiguousarray(
        W2.astype(NPBF16).reshape(NF, KO2, 128, M1, 128)
        .transpose(0, 2, 3, 1, 4))
    W3pm = np.ascontiguousarray(
        W3.astype(NPBF16).reshape(NF, KO2, 128, C).transpose(0, 2, 1, 3))
    in_maps = []
    for core in range(NCORES):
        es = expert_of[core]
        im = {
            "w1": W1pm[es],
            "w2": W2pm[es],
            "w3": W3pm[es],
            "b1": np.ascontiguousarray(b1[es]),
            "b2": np.ascontiguousarray(b2[es]),
            "b3": np.ascontiguousarray(b3[es]),
        }
        for si, s in enumerate(sizes):
            xb = xpad[rowmap[core][si].reshape(-1)]  # [-1 -> zero row]
            im[f"xb{si}"] = np.ascontiguousarray(
                xb.reshape(ccount[s], s, KO1, 128).transpose(0, 3, 2, 1))
        in_maps.append(im)

    nc = _build_program(tuple(prof), KO1, KO2, H, C)
    kwargs = {}
    if trace:
        kwargs = dict(trace=True,
                      trace_cores=trace_cores or list(range(NCORES)))
    res = run_bass_kernel_spmd(nc, in_maps, list(range(NCORES)), **kwargs)

    # --- scatter back
    for core in range(NCORES):
        for si, s in enumerate(sizes):
            ob = np.asarray(res.results[core][f"outb{si}"])  # [n, C, s]
            rows = ob.transpose(0, 2, 1).reshape(-1, C)
            ids = rowmap[core][si].reshape(-1)
            valid = ids >= 0
            out_full[ids[valid]] = rows[valid]

    return (out_full, res) if trace else out_full


def kernel(**inputs):
    return _execute(inputs)
